# revision 42
# baseline (speedup 1.0000x reference)
"""Trainium2 Bass kernel for nn_Encoder_8718783611479.

Reference computation (per the original nn.Module):
    xt = transpose(x, (0,3,1,2)).reshape(B, T, 180)          # B=2048, T=240
    gates = xt @ W_ih.T + b_ih + b_hh                        # (B, T, 360)
    i, f, g, o = split(gates, 4)                             # f unused (c0=0)
    c = sigmoid(i) * tanh(g)
    h = sigmoid(o) * tanh(c)
    out = sigmoid(h)                                         # (B, T, 90) f32

Design notes:
  * The LSTMCell is stateless per timestep -> one big GEMM over (B*T, 180).
  * x[b] viewed as (180, 240) is ALREADY the transposed stationary operand
    (lhsT = [K, M]) the tensor engine wants; the reference's transpose is
    absorbed into the matmul for free.
  * The f gate is dead; only 270 of 360 gate columns are computed.
  * sigmoid(z) = (1 + tanh(z/2)) / 2, so one unified ACT pass
    T = tanh(0.5 * G) over the gate columns serves both the sigmoid
    (i, o) and tanh (g) gates -- the g columns' weights/bias are pre-scaled
    by 2 on the host so tanh(0.5 * 2g) = tanh(g).
  * Bias is folded into the matmul via an appended ones-row (K: 90 + 91).
  * The whole post-tanh elementwise chain runs as TWO fused custom-DVE ops
    (8-stage datapath, 1 elem/cycle each):
      op1: U   = tanhpoly5((1+Ti)*Tg)        [= tanh(c), |C|<=2]
      op2: out = 0.5 + Hh*(a0+a1*Hh^2),  Hh = (1+To)*U   [= sigmoid(h)]
  * To balance ACT vs DVE, a fraction of the o-gate tanh is computed by a
    third fused DVE op (deg-7 odd poly, 4th coeff via C3/Src1 spill) that
    reads the PSUM gates directly; ACT covers only [i,g] for those groups.
  * Host repacks x into two planar [row, batch*col] bf16 arrays so every
    DMA descriptor is a >=512B contiguous run; the device emits output in
    partition-major [128, tile, 90] layout and the host un-permutes.

Sharding: pure data parallel, batch 2048 -> 8 cores x 256.
"""

import threading

import numpy as np
import ml_dtypes

import concourse.bass as bass
import concourse.mybir as mybir
import concourse.tile as tile
from concourse.vector_clock import ScopedClock
from concourse.bass_utils import run_bass_kernel_spmd

BF16 = ml_dtypes.bfloat16

# ---- problem constants (hardcoded per contract) ----
B, T, D_IN, H = 2048, 240, 180, 90
N_CORES = 8
BC = B // N_CORES          # 256 batches per core
NG = 270                   # live gate columns (i, g, o)

# ---- tiling config ----
NB = 16                    # batches per super-tile
ROWS = NB * T              # 3840 rows per super-tile
MT = ROWS // 128           # 30 m-tiles of 128 rows per super-tile
NSUP = BC // NB            # super-tiles per core
NMT = BC * T // 128        # 480 m-tiles per core

# ---- polynomial coefficients (minimax, see design notes) ----
# tanh(2y) ~= y*(P0 + P1*y^2) on |y|<=0.5, err < 4.6e-3 (3-op DVE variant)
_P3 = (1.95118309, -1.74855998)
# tanh(2y) ~= y*(P0 + P1*y^2 + P2*y^4) on |y|<=0.5, err < 3.9e-4
_P5 = (1.99432488, -2.46386409, 2.32965461)
# sigma(h)-0.5 ~= h*(Q0 + Q1*h^2) on |h|<=0.78, err < 3.5e-5
_Q = (0.24978061, -0.01935041)


# ---------------------------------------------------------------------------
# Walrus single-sync-wait workaround (installed walrus accepts at most ONE
# sync wait per ISA instruction; hoist overflow waits onto same-engine NOPs).
# ---------------------------------------------------------------------------
def _patched_lower_ordered_insts(self, ordered):
    for bb_name, insts in ordered.items():
        new = []
        for inst in insts:
            si = getattr(inst, "sync_info", None)
            if si is not None and len(si.on_wait) > 1:
                waits = list(si.on_wait)
                for w in waits[:-1]:
                    nop = mybir.InstNoOp(
                        name=self.nc.get_next_instruction_name(),
                        sync_info=mybir.SyncInfo(on_wait=[w], on_update=[]),
                        bass_nofuse=True,
                        engine=inst.engine,
                    )
                    new.append(nop)
                inst.sync_info = mybir.SyncInfo(
                    on_wait=[waits[-1]], on_update=list(si.on_update)
                )
            new.append(inst)
        insts[:] = new
    return _orig_lower_ordered_insts(self, ordered)


def _patched_drain_and_barrier(self, tick_clock, wait_clock):
    drain_inst = self.nc.sync.drain()
    wait_clock.add_sem_waits(
        drain_inst.ins, ScopedClock({None: tick_clock.global_clock})
    )
    si = drain_inst.ins.sync_info
    if si is not None and len(si.on_wait) > 1:
        waits = list(si.on_wait)
        drain_inst.ins.sync_info = mybir.SyncInfo(
            on_wait=waits[:1], on_update=list(si.on_update)
        )
        for i in range(1, len(waits)):
            nop = self.nc.sync.nop(nofuse=True)
            nop.ins.sync_info = mybir.SyncInfo(on_wait=[waits[i]], on_update=[])
    self.nc.all_engine_barrier()
    assert self.sems is not None
    popped = self.nc._tile_sem_poison_stack.pop()
    assert popped is self._sem_poison
    self.nc.clear_and_free_semaphores(list(self.sems.allocated().values()))
    self.nc.all_engine_barrier()


if not getattr(tile.TileContext, "_single_wait_patched", False):
    tile.TileContext._orig_lower_unpatched = tile.TileContext._lower_ordered_insts
    tile.TileContext._lower_ordered_insts = _patched_lower_ordered_insts
    tile.TileContext._drain_and_barrier = _patched_drain_and_barrier
    tile.TileContext._single_wait_patched = True
_orig_lower_ordered_insts = tile.TileContext._orig_lower_unpatched


def _groups_for(mt_j, gsz0):
    if gsz0 == 4:
        groups, left = [], mt_j
        while left >= 7 or left == 4:
            groups.append(4)
            left -= 4
        while left:
            groups.append(3)
            left -= 3
        return groups
    assert mt_j % gsz0 == 0
    return [gsz0] * (mt_j // gsz0)


def build_nc(
    out_dt="f16",
    bufs_x=3,
    bufs_t=3,
    bufs_ep=6,
    bufs_o=2,
    bufs_ps=2,
    ep_chunks=1,
    super_plan=None,
    gsz0=4,
    in_dma_chunks=2,
    tanh_dve_tiles=24,
    pool_c2=True,
    pool_out=True,
    tanh3=True,
    loop_repeat=None,
    w_dma_act=True,
    alloc_mode="stack",
    skip_ep=False,
    skip_act=False,
):
    f32 = mybir.dt.float32
    bf16 = mybir.dt.bfloat16
    fp8 = mybir.dt.float8e4
    odt = {"f32": f32, "bf16": bf16, "f16": mybir.dt.float16}[out_dt]
    FT = mybir.ActivationFunctionType

    nc = bass.Bass()
    # xq: x packed fp8 into two 91-row k-tiles (90 feature rows + a ones row
    # each); wq likewise holds W plus bias row fp8(b) in tile0 and the
    # residual b - fp8(b) in tile1, so the bias is exact to second order.
    xq = nc.declare_dram_parameter("xq", [91, 2, BC * T], fp8, isOutput=False)
    wq = nc.declare_dram_parameter("wq", [91, 2, NG], fp8, isOutput=False)
    # partition-major output: [partition, global m-tile, H]
    out = nc.declare_dram_parameter("out", [128, NMT, H], odt, isOutput=True)

    with tile.TileContext(nc, pool_alloc_mode=alloc_mode) as tc:
        with (
            tc.tile_pool(name="w", bufs=1) as wpool,
            tc.tile_pool(name="x", bufs=bufs_x) as xpool,
            tc.tile_pool(name="t", bufs=bufs_t) as tpool,
            tc.tile_pool(name="ep", bufs=bufs_ep) as eppool,
            tc.tile_pool(name="o", bufs=bufs_o) as opool,
            tc.tile_pool(name="ps", bufs=bufs_ps, space="PSUM") as pspool,
        ):
            wdma = nc.scalar if w_dma_act else nc.sync
            ws = wpool.tile([91, 2, NG], fp8)
            wdma.dma_start(ws[:], wq[:, :, :])

            if super_plan is None:
                plan = [(NB, ep_chunks)] * NSUP
            else:
                plan = super_plan
            assert sum(nb for nb, _ in plan) == BC
            max_rows = max(nb for nb, _ in plan) * T
            max_mt = max_rows // 128
            import contextlib

            loop_ctx = (
                tc.For_i(0, loop_repeat, 1)
                if loop_repeat is not None
                else contextlib.nullcontext()
            )
            def emit_chunks(Tt, mt_j, mt_base, epc_j):
                # staged S = sigma(0.5*G') per class: Si=sigma(i), Sg=sigma(2g),
                # So=sigma(o). Then c/2 = (Sg-0.5)*Si, U = tanh(c),
                # h = So*U, out = sigma(h).
                for h2 in range(epc_j if not skip_ep else 0):
                    mlo = mt_j * h2 // epc_j
                    mhi = mt_j * (h2 + 1) // epc_j
                    F = (mhi - mlo) * H
                    Ts = Tt[:, :, mlo:mhi, :]
                    Si = Ts[:, 0].rearrange("p t c -> p (t c)")
                    Sg = Ts[:, 1].rearrange("p t c -> p (t c)")
                    So = Ts[:, 2].rearrange("p t c -> p (t c)")

                    c2 = eppool.tile([128, F], bf16, tag="ep")
                    if pool_c2:
                        # Pool cannot run scalar_tensor_tensor (walrus
                        # rejects); split: Pool does the subtract, DVE the
                        # multiply.
                        tg = eppool.tile([128, F], bf16, tag="ep")
                        nc.gpsimd.tensor_scalar_add(tg[:], Sg, -0.5)
                        nc.vector.tensor_mul(c2[:], tg[:], Si)
                    else:
                        nc.vector.scalar_tensor_tensor(
                            c2[:], Sg, 0.5, Si,
                            mybir.AluOpType.subtract, mybir.AluOpType.mult,
                        )
                    U = eppool.tile([128, F], bf16, tag="ep")
                    # split U = tanh(2*c2): ACT (tanh table, scale=2) on the
                    # first tiles, DVE odd minimax poly on the rest,
                    # balancing the two engines
                    nmt_c = mhi - mlo
                    tdt = tanh_dve_tiles
                    kD = min((nmt_c * tdt) // MT, nmt_c) if tdt else 0
                    fA = (nmt_c - kD) * H
                    if fA > 0:
                        nc.scalar.activation(
                            U[:, 0:fA], c2[:, 0:fA], FT.Tanh, scale=2.0
                        )
                    if kD > 0:
                        Cs = c2[:, fA:F]
                        FD = F - fA
                        v = eppool.tile([128, FD], bf16, tag="pv")
                        nc.vector.tensor_mul(v[:], Cs, Cs)
                        if tanh3:
                            w = eppool.tile([128, FD], bf16, tag="pv")
                            nc.vector.tensor_scalar(
                                w[:], v[:], _P3[1], _P3[0],
                                mybir.AluOpType.mult, mybir.AluOpType.add,
                            )
                            nc.vector.tensor_mul(U[:, fA:F], w[:], Cs)
                        else:
                            w = eppool.tile([128, FD], bf16, tag="pv")
                            nc.vector.tensor_scalar(
                                w[:], v[:], _P5[2], _P5[1],
                                mybir.AluOpType.mult, mybir.AluOpType.add,
                            )
                            w2 = eppool.tile([128, FD], bf16, tag="pv")
                            nc.vector.tensor_mul(w2[:], w[:], v[:])
                            w3 = eppool.tile([128, FD], bf16, tag="pv")
                            nc.vector.tensor_scalar_add(w3[:], w2[:], _P5[0])
                            nc.vector.tensor_mul(U[:, fA:F], w3[:], Cs)
                    hT = eppool.tile([128, F], bf16, tag="ep")
                    nc.vector.tensor_mul(hT[:], So, U[:])

                    # sigma(h) = 0.5 + h*(Q0 + Q1*h^2), err < 3.5e-5
                    W2 = eppool.tile([128, F], bf16, tag="ep")
                    nc.vector.tensor_mul(W2[:], hT[:], hT[:])
                    V = eppool.tile([128, F], bf16, tag="ep")
                    nc.vector.tensor_scalar(
                        V[:], W2[:], _Q[1], _Q[0],
                        mybir.AluOpType.mult, mybir.AluOpType.add,
                    )
                    Z = eppool.tile([128, F], bf16, tag="ep")
                    nc.vector.tensor_mul(Z[:], V[:], hT[:])
                    OUT = opool.tile([128, F], odt)
                    (nc.gpsimd if pool_out else nc.vector).tensor_scalar_add(
                        OUT[:], Z[:], 0.5
                    )
                    nc.sync.dma_start(
                        out[:, mt_base + mlo : mt_base + mhi, :],
                        OUT[:].rearrange("p (t c) -> p t c", c=H),
                    )

            with loop_ctx:
                c0 = 0
                mt_base = 0
                # one-super software pipeline: emit super j's staging
                # (DMA + matmul + ACT T-pass), then super j-1's elementwise
                # chunks. In-order engines then never head-of-line block:
                # ACT streams [T(j), U(j-1)], DVE streams chunk ops whose
                # ACT inputs landed a super ago.
                staged_prev = None
                for j, (nb_j, epc_j) in enumerate(plan):
                    rows_j = nb_j * T
                    mt_j = rows_j // 128
                    xc = xpool.tile([91, 2, max_rows], fp8, tag="xc")
                    ndc = max(1, min(in_dma_chunks, mt_j))
                    for d in range(ndc):
                        rl = rows_j * d // ndc
                        rh = rows_j * (d + 1) // ndc
                        nc.sync.dma_start(
                            xc[:, :, rl:rh], xq[:, :, c0 + rl : c0 + rh]
                        )

                    # gate-class-major staging of tanh(0.5*G): [gate, mtile, col]
                    Tt = tpool.tile([128, 3, max_mt, H], bf16, tag="Tt")
                    groups = _groups_for(mt_j, gsz0)
                    m0 = 0
                    for gsz in groups:
                        ps = pspool.tile([128, gsz0, 512], f32, tag="ps")
                        for t in range(gsz):
                            m = m0 + t
                            nc.tensor.matmul(
                                ps[:, t, 0:NG],
                                xc[:, :, 128 * m : 128 * (m + 1)],
                                ws[:],
                                start=True,
                                stop=True,
                                perf_mode=mybir.MatmulPerfMode.DoubleRow,
                            )
                        if skip_act:
                            nc.vector.tensor_copy(
                                Tt[:, 0, m0 : m0 + gsz, :],
                                ps[:, 0:gsz, 0:H],
                            )
                        else:
                            nc.scalar.activation(
                                Tt[:, :, m0 : m0 + gsz, :].rearrange(
                                    "p g t c -> p t g c"
                                ),
                                ps[:, 0:gsz, 0:NG].rearrange(
                                    "p t (g c) -> p t g c", g=3
                                ),
                                FT.Sigmoid,
                                scale=0.5,
                            )
                        m0 += gsz

                    if staged_prev is not None:
                        emit_chunks(*staged_prev)
                    staged_prev = (Tt, mt_j, mt_base, epc_j)
                    c0 += rows_j
                    mt_base += mt_j
                if staged_prev is not None:
                    emit_chunks(*staged_prev)
    return nc


_cache = threading.local()


DEFAULT_PLAN = [(8, 1)] * 2 + [(16, 1)] * 14 + [(8, 1)] * 2
DEFAULT_CFG = dict(
    super_plan=DEFAULT_PLAN, tanh_dve_tiles=18, pool_c2=True, pool_out=True,
    bufs_ep=10, bufs_o=3,
)


def _get_nc():
    nc = getattr(_cache, "nc", None)
    if nc is None:
        nc = build_nc(**DEFAULT_CFG)
        _cache.nc = nc
    return nc


FP8 = ml_dtypes.float8_e4m3


def _prep_inputs(x, W_ih, W_hh, b_ih, b_hh):
    x = np.asarray(x, dtype=np.float32)
    W = np.asarray(W_ih, dtype=np.float32)
    b = np.asarray(b_ih, dtype=np.float32) + np.asarray(b_hh, dtype=np.float32)
    # gate order [i, g, o]; prescale [2, 4, 2] so one sigma(0.5*G') ACT pass
    # yields [sigma(i), sigma(2g), sigma(o)] (tanh(g) = 2*sigma(2g) - 1)
    W_eff = np.concatenate(
        [2.0 * W[0:90], 4.0 * W[180:270], 2.0 * W[270:360]], axis=0
    )
    b_eff = np.concatenate(
        [2.0 * b[0:90], 4.0 * b[180:270], 2.0 * b[270:360]], axis=0
    )
    WT = W_eff.T                                  # (180, 270)
    wqv = np.zeros((91, 2, NG), dtype=FP8)
    wqv[0:90, 0] = WT[0:90].astype(FP8)
    wqv[0:90, 1] = WT[90:180].astype(FP8)
    b1 = b_eff.astype(FP8)
    wqv[90, 0] = b1
    wqv[90, 1] = (b_eff - b1.astype(np.float32)).astype(FP8)

    xr = x.reshape(B, D_IN, T).astype(FP8)        # (2048, 180, 240)
    # per-core pack: [k-row, tile, batch*col] planar, ones row at k=90
    xqs = []
    for c in range(N_CORES):
        xc = xr[c * BC : (c + 1) * BC]            # (256, 180, 240)
        xqv = np.empty((91, 2, BC * T), dtype=FP8)
        xqv[0:90, 0] = np.ascontiguousarray(
            xc[:, 0:90, :].transpose(1, 0, 2)
        ).reshape(90, BC * T)
        xqv[0:90, 1] = np.ascontiguousarray(
            xc[:, 90:180, :].transpose(1, 0, 2)
        ).reshape(90, BC * T)
        xqv[90, :] = 1.0
        xqs.append(xqv)
    return xqs, wqv


def kernel(x, W_ih, W_hh, b_ih, b_hh, _trace=False):
    xqs, wqv = _prep_inputs(x, W_ih, W_hh, b_ih, b_hh)
    nc = _get_nc()
    in_maps = [{"xq": xqs[c], "wq": wqv} for c in range(N_CORES)]
    res = run_bass_kernel_spmd(nc, in_maps, list(range(N_CORES)), trace=_trace)
    # device out: [128, NMT, H], row (128*g + p) -> un-permute on host
    outs = []
    for c in range(N_CORES):
        o = res.results[c]["out"].astype(np.float32)
        outs.append(o.transpose(1, 0, 2).reshape(BC, T, H))
    if _trace:
        kernel._last_results = res
    return np.concatenate(outs, axis=0)


# revision 43
# speedup vs baseline: 6.7537x; 6.7537x over previous
"""Trainium2 Bass kernel for nn_Encoder_8718783611479.

Reference computation (per the original nn.Module):
    xt = transpose(x, (0,3,1,2)).reshape(B, T, 180)          # B=2048, T=240
    gates = xt @ W_ih.T + b_ih + b_hh                        # (B, T, 360)
    i, f, g, o = split(gates, 4)                             # f unused (c0=0)
    c = sigmoid(i) * tanh(g)
    h = sigmoid(o) * tanh(c)
    out = sigmoid(h)                                         # (B, T, 90) f32

Design notes:
  * The LSTMCell is stateless per timestep -> one big GEMM over (B*T, 180).
  * x[b] viewed as (180, 240) is ALREADY the transposed stationary operand
    (lhsT = [K, M]) the tensor engine wants; the reference's transpose is
    absorbed into the matmul for free.
  * The f gate is dead; only 270 of 360 gate columns are computed.
  * sigmoid(z) = (1 + tanh(z/2)) / 2, so one unified ACT pass
    T = tanh(0.5 * G) over the gate columns serves both the sigmoid
    (i, o) and tanh (g) gates -- the g columns' weights/bias are pre-scaled
    by 2 on the host so tanh(0.5 * 2g) = tanh(g).
  * Bias is folded into the matmul via an appended ones-row (K: 90 + 91).
  * The whole post-tanh elementwise chain runs as TWO fused custom-DVE ops
    (8-stage datapath, 1 elem/cycle each):
      op1: U   = tanhpoly5((1+Ti)*Tg)        [= tanh(c), |C|<=2]
      op2: out = 0.5 + Hh*(a0+a1*Hh^2),  Hh = (1+To)*U   [= sigmoid(h)]
  * To balance ACT vs DVE, a fraction of the o-gate tanh is computed by a
    third fused DVE op (deg-7 odd poly, 4th coeff via C3/Src1 spill) that
    reads the PSUM gates directly; ACT covers only [i,g] for those groups.
  * Host repacks x into two planar [row, batch*col] bf16 arrays so every
    DMA descriptor is a >=512B contiguous run; the device emits output in
    partition-major [128, tile, 90] layout and the host un-permutes.

Sharding: pure data parallel, batch 2048 -> 8 cores x 256.
"""

import threading

import numpy as np
import ml_dtypes

import concourse.bass as bass
import concourse.mybir as mybir
import concourse.tile as tile
from concourse.vector_clock import ScopedClock
from concourse.bass_utils import run_bass_kernel_spmd

BF16 = ml_dtypes.bfloat16

# ---- problem constants (hardcoded per contract) ----
B, T, D_IN, H = 2048, 240, 180, 90
N_CORES = 8
BC = B // N_CORES          # 256 batches per core
NG = 270                   # live gate columns (i, g, o)

# ---- tiling config ----
NB = 16                    # batches per super-tile
ROWS = NB * T              # 3840 rows per super-tile
MT = ROWS // 128           # 30 m-tiles of 128 rows per super-tile
NSUP = BC // NB            # super-tiles per core
NMT = BC * T // 128        # 480 m-tiles per core

# ---- polynomial coefficients (minimax, see design notes) ----
# tanh(2y) ~= y*(P0 + P1*y^2) on |y|<=0.5, err < 4.6e-3 (3-op DVE variant)
_P3 = (1.95118309, -1.74855998)
# tanh(2y) ~= y*(P0 + P1*y^2 + P2*y^4) on |y|<=0.5, err < 3.9e-4
_P5 = (1.99432488, -2.46386409, 2.32965461)
# sigma(h)-0.5 ~= h*(Q0 + Q1*h^2) on |h|<=0.78, err < 3.5e-5
_Q = (0.24978061, -0.01935041)


# ---------------------------------------------------------------------------
# Walrus single-sync-wait workaround (installed walrus accepts at most ONE
# sync wait per ISA instruction; hoist overflow waits onto same-engine NOPs).
# ---------------------------------------------------------------------------
def _patched_lower_ordered_insts(self, ordered):
    for bb_name, insts in ordered.items():
        new = []
        for inst in insts:
            si = getattr(inst, "sync_info", None)
            if si is not None and len(si.on_wait) > 1:
                waits = list(si.on_wait)
                for w in waits[:-1]:
                    nop = mybir.InstNoOp(
                        name=self.nc.get_next_instruction_name(),
                        sync_info=mybir.SyncInfo(on_wait=[w], on_update=[]),
                        bass_nofuse=True,
                        engine=inst.engine,
                    )
                    new.append(nop)
                inst.sync_info = mybir.SyncInfo(
                    on_wait=[waits[-1]], on_update=list(si.on_update)
                )
            new.append(inst)
        insts[:] = new
    return _orig_lower_ordered_insts(self, ordered)


def _patched_drain_and_barrier(self, tick_clock, wait_clock):
    drain_inst = self.nc.sync.drain()
    wait_clock.add_sem_waits(
        drain_inst.ins, ScopedClock({None: tick_clock.global_clock})
    )
    si = drain_inst.ins.sync_info
    if si is not None and len(si.on_wait) > 1:
        waits = list(si.on_wait)
        drain_inst.ins.sync_info = mybir.SyncInfo(
            on_wait=waits[:1], on_update=list(si.on_update)
        )
        for i in range(1, len(waits)):
            nop = self.nc.sync.nop(nofuse=True)
            nop.ins.sync_info = mybir.SyncInfo(on_wait=[waits[i]], on_update=[])
    self.nc.all_engine_barrier()
    assert self.sems is not None
    popped = self.nc._tile_sem_poison_stack.pop()
    assert popped is self._sem_poison
    self.nc.clear_and_free_semaphores(list(self.sems.allocated().values()))
    self.nc.all_engine_barrier()


if not getattr(tile.TileContext, "_single_wait_patched", False):
    tile.TileContext._orig_lower_unpatched = tile.TileContext._lower_ordered_insts
    tile.TileContext._lower_ordered_insts = _patched_lower_ordered_insts
    tile.TileContext._drain_and_barrier = _patched_drain_and_barrier
    tile.TileContext._single_wait_patched = True
_orig_lower_ordered_insts = tile.TileContext._orig_lower_unpatched


def _groups_for(mt_j, gsz0):
    if gsz0 == 4:
        groups, left = [], mt_j
        while left >= 7 or left == 4:
            groups.append(4)
            left -= 4
        while left:
            groups.append(3)
            left -= 3
        return groups
    assert mt_j % gsz0 == 0
    return [gsz0] * (mt_j // gsz0)


def build_nc(
    out_dt="f16",
    bufs_x=3,
    bufs_t=3,
    bufs_ep=6,
    bufs_o=2,
    bufs_ps=2,
    ep_chunks=1,
    super_plan=None,
    gsz0=4,
    in_dma_chunks=2,
    tanh_dve_tiles=24,
    pool_c2=True,
    pool_out=True,
    tanh3=True,
    loop_repeat=None,
    w_dma_act=True,
    alloc_mode="stack",
    skip_ep=False,
    skip_act=False,
):
    f32 = mybir.dt.float32
    bf16 = mybir.dt.bfloat16
    fp8 = mybir.dt.float8e4
    odt = {"f32": f32, "bf16": bf16, "f16": mybir.dt.float16}[out_dt]
    FT = mybir.ActivationFunctionType

    nc = bass.Bass()
    # xq: x packed fp8 into two 91-row k-tiles (90 feature rows + a ones row
    # each); wq likewise holds W plus bias row fp8(b) in tile0 and the
    # residual b - fp8(b) in tile1, so the bias is exact to second order.
    xq = nc.declare_dram_parameter("xq", [91, 2, BC * T], fp8, isOutput=False)
    wq = nc.declare_dram_parameter("wq", [91, 2, NG], fp8, isOutput=False)
    # partition-major output: [partition, global m-tile, H]
    out = nc.declare_dram_parameter("out", [128, NMT, H], odt, isOutput=True)

    with tile.TileContext(nc, pool_alloc_mode=alloc_mode) as tc:
        with (
            tc.tile_pool(name="w", bufs=1) as wpool,
            tc.tile_pool(name="x", bufs=bufs_x) as xpool,
            tc.tile_pool(name="t", bufs=bufs_t) as tpool,
            tc.tile_pool(name="ep", bufs=bufs_ep) as eppool,
            tc.tile_pool(name="o", bufs=bufs_o) as opool,
            tc.tile_pool(name="ps", bufs=bufs_ps, space="PSUM") as pspool,
        ):
            wdma = nc.scalar if w_dma_act else nc.sync
            ws = wpool.tile([91, 2, NG], fp8)
            wdma.dma_start(ws[:], wq[:, :, :])

            if super_plan is None:
                plan = [(NB, ep_chunks)] * NSUP
            else:
                plan = super_plan
            assert sum(nb for nb, _ in plan) == BC
            max_rows = max(nb for nb, _ in plan) * T
            max_mt = max_rows // 128
            import contextlib

            loop_ctx = (
                tc.For_i(0, loop_repeat, 1)
                if loop_repeat is not None
                else contextlib.nullcontext()
            )
            def emit_chunks(Tt, mt_j, mt_base, epc_j):
                # staged S = sigma(0.5*G') per class: Si=sigma(i), Sg=sigma(2g),
                # So=sigma(o). Then c/2 = (Sg-0.5)*Si, U = tanh(c),
                # h = So*U, out = sigma(h).
                for h2 in range(epc_j if not skip_ep else 0):
                    mlo = mt_j * h2 // epc_j
                    mhi = mt_j * (h2 + 1) // epc_j
                    F = (mhi - mlo) * H
                    Ts = Tt[:, :, mlo:mhi, :]
                    Si = Ts[:, 0].rearrange("p t c -> p (t c)")
                    Sg = Ts[:, 1].rearrange("p t c -> p (t c)")
                    So = Ts[:, 2].rearrange("p t c -> p (t c)")

                    c2 = eppool.tile([128, F], bf16, tag="ep")
                    if pool_c2:
                        # Pool cannot run scalar_tensor_tensor (walrus
                        # rejects); split: Pool does the subtract, DVE the
                        # multiply.
                        tg = eppool.tile([128, F], bf16, tag="ep")
                        nc.gpsimd.tensor_scalar_add(tg[:], Sg, -0.5)
                        nc.vector.tensor_mul(c2[:], tg[:], Si)
                    else:
                        nc.vector.scalar_tensor_tensor(
                            c2[:], Sg, 0.5, Si,
                            mybir.AluOpType.subtract, mybir.AluOpType.mult,
                        )
                    U = eppool.tile([128, F], bf16, tag="ep")
                    # split U = tanh(2*c2): ACT (tanh table, scale=2) on the
                    # first tiles, DVE odd minimax poly on the rest,
                    # balancing the two engines
                    nmt_c = mhi - mlo
                    tdt = tanh_dve_tiles
                    kD = min((nmt_c * tdt) // MT, nmt_c) if tdt else 0
                    fA = (nmt_c - kD) * H
                    if fA > 0:
                        nc.scalar.activation(
                            U[:, 0:fA], c2[:, 0:fA], FT.Tanh, scale=2.0
                        )
                    if kD > 0:
                        Cs = c2[:, fA:F]
                        FD = F - fA
                        v = eppool.tile([128, FD], bf16, tag="pv")
                        nc.vector.tensor_mul(v[:], Cs, Cs)
                        if tanh3:
                            w = eppool.tile([128, FD], bf16, tag="pv")
                            nc.vector.tensor_scalar(
                                w[:], v[:], _P3[1], _P3[0],
                                mybir.AluOpType.mult, mybir.AluOpType.add,
                            )
                            nc.vector.tensor_mul(U[:, fA:F], w[:], Cs)
                        else:
                            w = eppool.tile([128, FD], bf16, tag="pv")
                            nc.vector.tensor_scalar(
                                w[:], v[:], _P5[2], _P5[1],
                                mybir.AluOpType.mult, mybir.AluOpType.add,
                            )
                            w2 = eppool.tile([128, FD], bf16, tag="pv")
                            nc.vector.tensor_mul(w2[:], w[:], v[:])
                            w3 = eppool.tile([128, FD], bf16, tag="pv")
                            nc.vector.tensor_scalar_add(w3[:], w2[:], _P5[0])
                            nc.vector.tensor_mul(U[:, fA:F], w3[:], Cs)
                    hT = eppool.tile([128, F], bf16, tag="ep")
                    nc.vector.tensor_mul(hT[:], So, U[:])

                    # sigma(h) = 0.5 + h*(Q0 + Q1*h^2), err < 3.5e-5
                    W2 = eppool.tile([128, F], bf16, tag="ep")
                    nc.vector.tensor_mul(W2[:], hT[:], hT[:])
                    V = eppool.tile([128, F], bf16, tag="ep")
                    nc.vector.tensor_scalar(
                        V[:], W2[:], _Q[1], _Q[0],
                        mybir.AluOpType.mult, mybir.AluOpType.add,
                    )
                    Z = eppool.tile([128, F], bf16, tag="ep")
                    nc.vector.tensor_mul(Z[:], V[:], hT[:])
                    OUT = opool.tile([128, F], odt)
                    (nc.gpsimd if pool_out else nc.vector).tensor_scalar_add(
                        OUT[:], Z[:], 0.5
                    )
                    nc.sync.dma_start(
                        out[:, mt_base + mlo : mt_base + mhi, :],
                        OUT[:].rearrange("p (t c) -> p t c", c=H),
                    )

            with loop_ctx:
                c0 = 0
                mt_base = 0
                # one-super software pipeline: emit super j's staging
                # (DMA + matmul + ACT T-pass), then super j-1's elementwise
                # chunks. In-order engines then never head-of-line block:
                # ACT streams [T(j), U(j-1)], DVE streams chunk ops whose
                # ACT inputs landed a super ago.
                staged_prev = None
                for j, (nb_j, epc_j) in enumerate(plan):
                    rows_j = nb_j * T
                    mt_j = rows_j // 128
                    xc = xpool.tile([91, 2, max_rows], fp8, tag="xc")
                    ndc = max(1, min(in_dma_chunks, mt_j))
                    for d in range(ndc):
                        rl = rows_j * d // ndc
                        rh = rows_j * (d + 1) // ndc
                        nc.sync.dma_start(
                            xc[:, :, rl:rh], xq[:, :, c0 + rl : c0 + rh]
                        )

                    # gate-class-major staging of tanh(0.5*G): [gate, mtile, col]
                    Tt = tpool.tile([128, 3, max_mt, H], bf16, tag="Tt")
                    groups = _groups_for(mt_j, gsz0)
                    m0 = 0
                    for gsz in groups:
                        ps = pspool.tile([128, gsz0, 512], f32, tag="ps")
                        for t in range(gsz):
                            m = m0 + t
                            nc.tensor.matmul(
                                ps[:, t, 0:NG],
                                xc[:, :, 128 * m : 128 * (m + 1)],
                                ws[:],
                                start=True,
                                stop=True,
                                perf_mode=mybir.MatmulPerfMode.DoubleRow,
                            )
                        if skip_act:
                            nc.vector.tensor_copy(
                                Tt[:, 0, m0 : m0 + gsz, :],
                                ps[:, 0:gsz, 0:H],
                            )
                        else:
                            nc.scalar.activation(
                                Tt[:, :, m0 : m0 + gsz, :].rearrange(
                                    "p g t c -> p t g c"
                                ),
                                ps[:, 0:gsz, 0:NG].rearrange(
                                    "p t (g c) -> p t g c", g=3
                                ),
                                FT.Sigmoid,
                                scale=0.5,
                            )
                        m0 += gsz

                    if staged_prev is not None:
                        emit_chunks(*staged_prev)
                    staged_prev = (Tt, mt_j, mt_base, epc_j)
                    c0 += rows_j
                    mt_base += mt_j
                if staged_prev is not None:
                    emit_chunks(*staged_prev)
    return nc


_cache = threading.local()


DEFAULT_PLAN = [(8, 1)] * 2 + [(16, 1)] * 14 + [(8, 1)] * 2
DEFAULT_CFG = dict(
    super_plan=DEFAULT_PLAN, tanh_dve_tiles=6, pool_c2=False, pool_out=False,
    bufs_ep=10, bufs_o=3,
)


def _get_nc():
    nc = getattr(_cache, "nc", None)
    if nc is None:
        nc = build_nc(**DEFAULT_CFG)
        _cache.nc = nc
    return nc


FP8 = ml_dtypes.float8_e4m3


def _prep_inputs(x, W_ih, W_hh, b_ih, b_hh):
    x = np.asarray(x, dtype=np.float32)
    W = np.asarray(W_ih, dtype=np.float32)
    b = np.asarray(b_ih, dtype=np.float32) + np.asarray(b_hh, dtype=np.float32)
    # gate order [i, g, o]; prescale [2, 4, 2] so one sigma(0.5*G') ACT pass
    # yields [sigma(i), sigma(2g), sigma(o)] (tanh(g) = 2*sigma(2g) - 1)
    W_eff = np.concatenate(
        [2.0 * W[0:90], 4.0 * W[180:270], 2.0 * W[270:360]], axis=0
    )
    b_eff = np.concatenate(
        [2.0 * b[0:90], 4.0 * b[180:270], 2.0 * b[270:360]], axis=0
    )
    WT = W_eff.T                                  # (180, 270)
    wqv = np.zeros((91, 2, NG), dtype=FP8)
    wqv[0:90, 0] = WT[0:90].astype(FP8)
    wqv[0:90, 1] = WT[90:180].astype(FP8)
    b1 = b_eff.astype(FP8)
    wqv[90, 0] = b1
    wqv[90, 1] = (b_eff - b1.astype(np.float32)).astype(FP8)

    xr = x.reshape(B, D_IN, T).astype(FP8)        # (2048, 180, 240)
    # per-core pack: [k-row, tile, batch*col] planar, ones row at k=90
    xqs = []
    for c in range(N_CORES):
        xc = xr[c * BC : (c + 1) * BC]            # (256, 180, 240)
        xqv = np.empty((91, 2, BC * T), dtype=FP8)
        xqv[0:90, 0] = np.ascontiguousarray(
            xc[:, 0:90, :].transpose(1, 0, 2)
        ).reshape(90, BC * T)
        xqv[0:90, 1] = np.ascontiguousarray(
            xc[:, 90:180, :].transpose(1, 0, 2)
        ).reshape(90, BC * T)
        xqv[90, :] = 1.0
        xqs.append(xqv)
    return xqs, wqv


def kernel(x, W_ih, W_hh, b_ih, b_hh, _trace=False):
    xqs, wqv = _prep_inputs(x, W_ih, W_hh, b_ih, b_hh)
    nc = _get_nc()
    in_maps = [{"xq": xqs[c], "wq": wqv} for c in range(N_CORES)]
    res = run_bass_kernel_spmd(nc, in_maps, list(range(N_CORES)), trace=_trace)
    # device out: [128, NMT, H], row (128*g + p) -> un-permute on host
    outs = []
    for c in range(N_CORES):
        o = res.results[c]["out"].astype(np.float32)
        outs.append(o.transpose(1, 0, 2).reshape(BC, T, H))
    if _trace:
        kernel._last_results = res
    return np.concatenate(outs, axis=0)


# revision 44
# speedup vs baseline: 10.6706x; 1.5800x over previous
"""Trainium2 Bass kernel for nn_Encoder_8718783611479.

Reference computation (per the original nn.Module):
    xt = transpose(x, (0,3,1,2)).reshape(B, T, 180)          # B=2048, T=240
    gates = xt @ W_ih.T + b_ih + b_hh                        # (B, T, 360)
    i, f, g, o = split(gates, 4)                             # f unused (c0=0)
    c = sigmoid(i) * tanh(g)
    h = sigmoid(o) * tanh(c)
    out = sigmoid(h)                                         # (B, T, 90) f32

Design notes:
  * The LSTMCell is stateless per timestep -> one big GEMM over (B*T, 180).
  * x[b] viewed as (180, 240) is ALREADY the transposed stationary operand
    (lhsT = [K, M]) the tensor engine wants; the reference's transpose is
    absorbed into the matmul for free.
  * The f gate is dead; only 270 of 360 gate columns are computed.
  * The GEMM runs in fp8 e4m3 with the DoubleRow perf mode (2 k-tiles of 91
    rows packed per matmul, 0.5 cycles/row): x and W are quantized to fp8;
    the bias rides in two ones-rows (fp8(b) and the residual b - fp8(b)),
    so bias is exact to second order. End-to-end fp8 error ~3e-3 rel.
  * One unified sigma ACT pass: host prescales weights [2, 4, 2] per gate
    class so S = sigma(0.5*G') = [sigma(i), sigma(2g), sigma(o)] in one
    table pass (tanh(g) = 2*sigma(2g) - 1).
  * Elementwise chain (DVE, bf16 2x/4x modes): c/2 = (Sg-0.5)*Si via one
    scalar_tensor_tensor; U = tanh(2*(c/2)) split between the ACT tanh
    table and a deg-3 odd DVE poly (tanh_dve_tiles tunes the balance);
    h = So*U; out = 0.5 + h*(Q0+Q1*h^2)  [= sigmoid(h), err 3.5e-5].
  * One-super software pipeline skew: super j's staging (DMA+matmul+ACT)
    is emitted before super j-1's elementwise chunks, so the in-order
    ACT/DVE streams never head-of-line block on each other.
  * GPSIMD (Pool) elementwise offload measured ~8x slower than the cost
    model on this HW -- pool_* flags exist but default off.
  * Host repacks x into a planar [k-row, tile, batch*col] fp8 array so
    every DMA descriptor is a long contiguous run; the device emits output
    in partition-major [128, tile, 90] f16 layout and the host un-permutes.

Sharding: pure data parallel, batch 2048 -> 8 cores x 256.
"""

import threading

import numpy as np
import ml_dtypes

import concourse.bass as bass
import concourse.mybir as mybir
import concourse.tile as tile
from concourse.vector_clock import ScopedClock
from concourse.bass_utils import run_bass_kernel_spmd

BF16 = ml_dtypes.bfloat16

# ---- problem constants (hardcoded per contract) ----
B, T, D_IN, H = 2048, 240, 180, 90
N_CORES = 8
BC = B // N_CORES          # 256 batches per core
NG = 270                   # live gate columns (i, g, o)

# ---- tiling config ----
NB = 16                    # batches per super-tile
ROWS = NB * T              # 3840 rows per super-tile
MT = ROWS // 128           # 30 m-tiles of 128 rows per super-tile
NSUP = BC // NB            # super-tiles per core
NMT = BC * T // 128        # 480 m-tiles per core

# ---- polynomial coefficients (minimax, see design notes) ----
# tanh(2y) ~= y*(P0 + P1*y^2) on |y|<=0.5, err < 4.6e-3 (3-op DVE variant)
_P3 = (1.95118309, -1.74855998)
# tanh(2y) ~= y*(P0 + P1*y^2 + P2*y^4) on |y|<=0.5, err < 3.9e-4
_P5 = (1.99432488, -2.46386409, 2.32965461)
# sigma(h)-0.5 ~= h*(Q0 + Q1*h^2) on |h|<=0.78, err < 3.5e-5
_Q = (0.24978061, -0.01935041)


# ---------------------------------------------------------------------------
# Walrus single-sync-wait workaround (installed walrus accepts at most ONE
# sync wait per ISA instruction; hoist overflow waits onto same-engine NOPs).
# ---------------------------------------------------------------------------
def _patched_lower_ordered_insts(self, ordered):
    for bb_name, insts in ordered.items():
        new = []
        for inst in insts:
            si = getattr(inst, "sync_info", None)
            if si is not None and len(si.on_wait) > 1:
                waits = list(si.on_wait)
                for w in waits[:-1]:
                    nop = mybir.InstNoOp(
                        name=self.nc.get_next_instruction_name(),
                        sync_info=mybir.SyncInfo(on_wait=[w], on_update=[]),
                        bass_nofuse=True,
                        engine=inst.engine,
                    )
                    new.append(nop)
                inst.sync_info = mybir.SyncInfo(
                    on_wait=[waits[-1]], on_update=list(si.on_update)
                )
            new.append(inst)
        insts[:] = new
    return _orig_lower_ordered_insts(self, ordered)


def _patched_drain_and_barrier(self, tick_clock, wait_clock):
    drain_inst = self.nc.sync.drain()
    wait_clock.add_sem_waits(
        drain_inst.ins, ScopedClock({None: tick_clock.global_clock})
    )
    si = drain_inst.ins.sync_info
    if si is not None and len(si.on_wait) > 1:
        waits = list(si.on_wait)
        drain_inst.ins.sync_info = mybir.SyncInfo(
            on_wait=waits[:1], on_update=list(si.on_update)
        )
        for i in range(1, len(waits)):
            nop = self.nc.sync.nop(nofuse=True)
            nop.ins.sync_info = mybir.SyncInfo(on_wait=[waits[i]], on_update=[])
    self.nc.all_engine_barrier()
    assert self.sems is not None
    popped = self.nc._tile_sem_poison_stack.pop()
    assert popped is self._sem_poison
    self.nc.clear_and_free_semaphores(list(self.sems.allocated().values()))
    self.nc.all_engine_barrier()


if not getattr(tile.TileContext, "_single_wait_patched", False):
    tile.TileContext._orig_lower_unpatched = tile.TileContext._lower_ordered_insts
    tile.TileContext._lower_ordered_insts = _patched_lower_ordered_insts
    tile.TileContext._drain_and_barrier = _patched_drain_and_barrier
    tile.TileContext._single_wait_patched = True
_orig_lower_ordered_insts = tile.TileContext._orig_lower_unpatched


def _groups_for(mt_j, gsz0):
    if gsz0 == 4:
        groups, left = [], mt_j
        while left >= 7 or left == 4:
            groups.append(4)
            left -= 4
        while left:
            groups.append(3)
            left -= 3
        return groups
    assert mt_j % gsz0 == 0
    return [gsz0] * (mt_j // gsz0)


def build_nc(
    out_dt="f16",
    bufs_x=3,
    bufs_t=3,
    bufs_ep=6,
    bufs_o=2,
    bufs_ps=2,
    ep_chunks=1,
    super_plan=None,
    gsz0=4,
    in_dma_chunks=2,
    tanh_dve_tiles=24,
    pool_c2=True,
    pool_out=True,
    tanh3=True,
    loop_repeat=None,
    w_dma_act=True,
    alloc_mode="stack",
    skip_ep=False,
    skip_act=False,
):
    f32 = mybir.dt.float32
    bf16 = mybir.dt.bfloat16
    fp8 = mybir.dt.float8e4
    odt = {"f32": f32, "bf16": bf16, "f16": mybir.dt.float16}[out_dt]
    FT = mybir.ActivationFunctionType

    nc = bass.Bass()
    # xq: x packed fp8 into two 91-row k-tiles (90 feature rows + a ones row
    # each); wq likewise holds W plus bias row fp8(b) in tile0 and the
    # residual b - fp8(b) in tile1, so the bias is exact to second order.
    xq = nc.declare_dram_parameter("xq", [91, 2, BC * T], fp8, isOutput=False)
    wq = nc.declare_dram_parameter("wq", [91, 2, NG], fp8, isOutput=False)
    # partition-major output: [partition, global m-tile, H]
    out = nc.declare_dram_parameter("out", [128, NMT, H], odt, isOutput=True)

    with tile.TileContext(nc, pool_alloc_mode=alloc_mode) as tc:
        with (
            tc.tile_pool(name="w", bufs=1) as wpool,
            tc.tile_pool(name="x", bufs=bufs_x) as xpool,
            tc.tile_pool(name="t", bufs=bufs_t) as tpool,
            tc.tile_pool(name="ep", bufs=bufs_ep) as eppool,
            tc.tile_pool(name="o", bufs=bufs_o) as opool,
            tc.tile_pool(name="ps", bufs=bufs_ps, space="PSUM") as pspool,
        ):
            wdma = nc.scalar if w_dma_act else nc.sync
            ws = wpool.tile([91, 2, NG], fp8)
            wdma.dma_start(ws[:], wq[:, :, :])

            if super_plan is None:
                plan = [(NB, ep_chunks)] * NSUP
            else:
                plan = super_plan
            assert sum(nb for nb, _ in plan) == BC
            max_rows = max(nb for nb, _ in plan) * T
            max_mt = max_rows // 128
            import contextlib

            loop_ctx = (
                tc.For_i(0, loop_repeat, 1)
                if loop_repeat is not None
                else contextlib.nullcontext()
            )
            def emit_chunks(Tt, mt_j, mt_base, epc_j):
                # staged S = sigma(0.5*G') per class: Si=sigma(i), Sg=sigma(2g),
                # So=sigma(o). Then c/2 = (Sg-0.5)*Si, U = tanh(c),
                # h = So*U, out = sigma(h).
                for h2 in range(epc_j if not skip_ep else 0):
                    mlo = mt_j * h2 // epc_j
                    mhi = mt_j * (h2 + 1) // epc_j
                    F = (mhi - mlo) * H
                    Ts = Tt[:, :, mlo:mhi, :]
                    Si = Ts[:, 0].rearrange("p t c -> p (t c)")
                    Sg = Ts[:, 1].rearrange("p t c -> p (t c)")
                    So = Ts[:, 2].rearrange("p t c -> p (t c)")

                    c2 = eppool.tile([128, F], bf16, tag="ep")
                    if pool_c2:
                        # Pool cannot run scalar_tensor_tensor (walrus
                        # rejects); split: Pool does the subtract, DVE the
                        # multiply.
                        tg = eppool.tile([128, F], bf16, tag="ep")
                        nc.gpsimd.tensor_scalar_add(tg[:], Sg, -0.5)
                        nc.vector.tensor_mul(c2[:], tg[:], Si)
                    else:
                        nc.vector.scalar_tensor_tensor(
                            c2[:], Sg, 0.5, Si,
                            mybir.AluOpType.subtract, mybir.AluOpType.mult,
                        )
                    U = eppool.tile([128, F], bf16, tag="ep")
                    # split U = tanh(2*c2): ACT (tanh table, scale=2) on the
                    # first tiles, DVE odd minimax poly on the rest,
                    # balancing the two engines
                    nmt_c = mhi - mlo
                    tdt = tanh_dve_tiles
                    kD = min((nmt_c * tdt) // MT, nmt_c) if tdt else 0
                    fA = (nmt_c - kD) * H
                    if fA > 0:
                        nc.scalar.activation(
                            U[:, 0:fA], c2[:, 0:fA], FT.Tanh, scale=2.0
                        )
                    if kD > 0:
                        Cs = c2[:, fA:F]
                        FD = F - fA
                        v = eppool.tile([128, FD], bf16, tag="pv")
                        nc.vector.tensor_mul(v[:], Cs, Cs)
                        if tanh3:
                            w = eppool.tile([128, FD], bf16, tag="pv")
                            nc.vector.tensor_scalar(
                                w[:], v[:], _P3[1], _P3[0],
                                mybir.AluOpType.mult, mybir.AluOpType.add,
                            )
                            nc.vector.tensor_mul(U[:, fA:F], w[:], Cs)
                        else:
                            w = eppool.tile([128, FD], bf16, tag="pv")
                            nc.vector.tensor_scalar(
                                w[:], v[:], _P5[2], _P5[1],
                                mybir.AluOpType.mult, mybir.AluOpType.add,
                            )
                            w2 = eppool.tile([128, FD], bf16, tag="pv")
                            nc.vector.tensor_mul(w2[:], w[:], v[:])
                            w3 = eppool.tile([128, FD], bf16, tag="pv")
                            nc.vector.tensor_scalar_add(w3[:], w2[:], _P5[0])
                            nc.vector.tensor_mul(U[:, fA:F], w3[:], Cs)
                    hT = eppool.tile([128, F], bf16, tag="ep")
                    nc.vector.tensor_mul(hT[:], So, U[:])

                    # sigma(h) = 0.5 + h*(Q0 + Q1*h^2), err < 3.5e-5
                    W2 = eppool.tile([128, F], bf16, tag="ep")
                    nc.vector.tensor_mul(W2[:], hT[:], hT[:])
                    V = eppool.tile([128, F], bf16, tag="ep")
                    nc.vector.tensor_scalar(
                        V[:], W2[:], _Q[1], _Q[0],
                        mybir.AluOpType.mult, mybir.AluOpType.add,
                    )
                    Z = eppool.tile([128, F], bf16, tag="ep")
                    nc.vector.tensor_mul(Z[:], V[:], hT[:])
                    OUT = opool.tile([128, F], odt)
                    (nc.gpsimd if pool_out else nc.vector).tensor_scalar_add(
                        OUT[:], Z[:], 0.5
                    )
                    nc.sync.dma_start(
                        out[:, mt_base + mlo : mt_base + mhi, :],
                        OUT[:].rearrange("p (t c) -> p t c", c=H),
                    )

            with loop_ctx:
                c0 = 0
                mt_base = 0
                # one-super software pipeline: emit super j's staging
                # (DMA + matmul + ACT T-pass), then super j-1's elementwise
                # chunks. In-order engines then never head-of-line block:
                # ACT streams [T(j), U(j-1)], DVE streams chunk ops whose
                # ACT inputs landed a super ago.
                staged_prev = None
                for j, (nb_j, epc_j) in enumerate(plan):
                    rows_j = nb_j * T
                    mt_j = rows_j // 128
                    xc = xpool.tile([91, 2, max_rows], fp8, tag="xc")
                    ndc = max(1, min(in_dma_chunks, mt_j))
                    for d in range(ndc):
                        rl = rows_j * d // ndc
                        rh = rows_j * (d + 1) // ndc
                        nc.sync.dma_start(
                            xc[:, :, rl:rh], xq[:, :, c0 + rl : c0 + rh]
                        )

                    # gate-class-major staging of tanh(0.5*G): [gate, mtile, col]
                    Tt = tpool.tile([128, 3, max_mt, H], bf16, tag="Tt")
                    groups = _groups_for(mt_j, gsz0)
                    m0 = 0
                    for gsz in groups:
                        ps = pspool.tile([128, gsz0, 512], f32, tag="ps")
                        for t in range(gsz):
                            m = m0 + t
                            nc.tensor.matmul(
                                ps[:, t, 0:NG],
                                xc[:, :, 128 * m : 128 * (m + 1)],
                                ws[:],
                                start=True,
                                stop=True,
                                perf_mode=mybir.MatmulPerfMode.DoubleRow,
                            )
                        if skip_act:
                            nc.vector.tensor_copy(
                                Tt[:, 0, m0 : m0 + gsz, :],
                                ps[:, 0:gsz, 0:H],
                            )
                        else:
                            nc.scalar.activation(
                                Tt[:, :, m0 : m0 + gsz, :].rearrange(
                                    "p g t c -> p t g c"
                                ),
                                ps[:, 0:gsz, 0:NG].rearrange(
                                    "p t (g c) -> p t g c", g=3
                                ),
                                FT.Sigmoid,
                                scale=0.5,
                            )
                        m0 += gsz

                    if staged_prev is not None:
                        emit_chunks(*staged_prev)
                    staged_prev = (Tt, mt_j, mt_base, epc_j)
                    c0 += rows_j
                    mt_base += mt_j
                if staged_prev is not None:
                    emit_chunks(*staged_prev)
    return nc


_cache = threading.local()


DEFAULT_PLAN = [(8, 1)] * 2 + [(16, 1)] * 14 + [(8, 1)] * 2
DEFAULT_CFG = dict(
    super_plan=DEFAULT_PLAN, tanh_dve_tiles=6, pool_c2=False, pool_out=False,
    bufs_ep=10, bufs_o=3,
)


def _get_nc():
    nc = getattr(_cache, "nc", None)
    if nc is None:
        nc = build_nc(**DEFAULT_CFG)
        _cache.nc = nc
    return nc


FP8 = ml_dtypes.float8_e4m3


def _prep_inputs(x, W_ih, W_hh, b_ih, b_hh):
    x = np.asarray(x, dtype=np.float32)
    W = np.asarray(W_ih, dtype=np.float32)
    b = np.asarray(b_ih, dtype=np.float32) + np.asarray(b_hh, dtype=np.float32)
    # gate order [i, g, o]; prescale [2, 4, 2] so one sigma(0.5*G') ACT pass
    # yields [sigma(i), sigma(2g), sigma(o)] (tanh(g) = 2*sigma(2g) - 1)
    W_eff = np.concatenate(
        [2.0 * W[0:90], 4.0 * W[180:270], 2.0 * W[270:360]], axis=0
    )
    b_eff = np.concatenate(
        [2.0 * b[0:90], 4.0 * b[180:270], 2.0 * b[270:360]], axis=0
    )
    WT = W_eff.T                                  # (180, 270)
    wqv = np.zeros((91, 2, NG), dtype=FP8)
    wqv[0:90, 0] = WT[0:90].astype(FP8)
    wqv[0:90, 1] = WT[90:180].astype(FP8)
    b1 = b_eff.astype(FP8)
    wqv[90, 0] = b1
    wqv[90, 1] = (b_eff - b1.astype(np.float32)).astype(FP8)

    xr = x.reshape(B, D_IN, T).astype(FP8)        # (2048, 180, 240)
    # per-core pack: [k-row, tile, batch*col] planar, ones row at k=90
    xqs = []
    for c in range(N_CORES):
        xc = xr[c * BC : (c + 1) * BC]            # (256, 180, 240)
        xqv = np.empty((91, 2, BC * T), dtype=FP8)
        xqv[0:90, 0] = np.ascontiguousarray(
            xc[:, 0:90, :].transpose(1, 0, 2)
        ).reshape(90, BC * T)
        xqv[0:90, 1] = np.ascontiguousarray(
            xc[:, 90:180, :].transpose(1, 0, 2)
        ).reshape(90, BC * T)
        xqv[90, :] = 1.0
        xqs.append(xqv)
    return xqs, wqv


def kernel(x, W_ih, W_hh, b_ih, b_hh, _trace=False):
    xqs, wqv = _prep_inputs(x, W_ih, W_hh, b_ih, b_hh)
    nc = _get_nc()
    in_maps = [{"xq": xqs[c], "wq": wqv} for c in range(N_CORES)]
    res = run_bass_kernel_spmd(nc, in_maps, list(range(N_CORES)), trace=_trace)
    # device out: [128, NMT, H], row (128*g + p) -> un-permute on host
    outs = []
    for c in range(N_CORES):
        o = res.results[c]["out"].astype(np.float32)
        outs.append(o.transpose(1, 0, 2).reshape(BC, T, H))
    if _trace:
        kernel._last_results = res
    return np.concatenate(outs, axis=0)


# revision 47
# speedup vs baseline: 14.4114x; 1.3506x over previous
"""Trainium2 Bass kernel for nn_Encoder_8718783611479.

Reference computation (per the original nn.Module):
    xt = transpose(x, (0,3,1,2)).reshape(B, T, 180)          # B=2048, T=240
    gates = xt @ W_ih.T + b_ih + b_hh                        # (B, T, 360)
    i, f, g, o = split(gates, 4)                             # f unused (c0=0)
    c = sigmoid(i) * tanh(g)
    h = sigmoid(o) * tanh(c)
    out = sigmoid(h)                                         # (B, T, 90) f32

Design notes:
  * The LSTMCell is stateless per timestep -> one big GEMM over (B*T, 180).
  * x[b] viewed as (180, 240) is ALREADY the transposed stationary operand
    (lhsT = [K, M]) the tensor engine wants; the reference's transpose is
    absorbed into the matmul for free.
  * The f gate is dead; only 270 of 360 gate columns are computed.
  * The GEMM runs in fp8 e4m3 with the DoubleRow perf mode (2 k-tiles of 91
    rows packed per matmul, 0.5 cycles/row): x and W are quantized to fp8;
    the bias rides in two ones-rows (fp8(b) and the residual b - fp8(b)),
    so bias is exact to second order. End-to-end fp8 error ~3e-3 rel.
  * One unified sigma ACT pass: host prescales weights [2, 4, 2] per gate
    class so S = sigma(0.5*G') = [sigma(i), sigma(2g), sigma(o)] in one
    table pass (tanh(g) = 2*sigma(2g) - 1).
  * Elementwise chain (DVE, bf16 2x/4x modes): c/2 = (Sg-0.5)*Si via one
    scalar_tensor_tensor; U = tanh(2*(c/2)) split between the ACT tanh
    table and a deg-3 odd DVE poly (tanh_dve_tiles tunes the balance);
    h = So*U; out = 0.5 + h*(Q0+Q1*h^2)  [= sigmoid(h), err 3.5e-5].
  * One-super software pipeline skew: super j's staging (DMA+matmul+ACT)
    is emitted before super j-1's elementwise chunks, so the in-order
    ACT/DVE streams never head-of-line block on each other.
  * GPSIMD (Pool) elementwise offload measured ~8x slower than the cost
    model on this HW -- pool_* flags exist but default off.
  * Host repacks x into a planar [k-row, tile, batch*col] fp8 array so
    every DMA descriptor is a long contiguous run; the device emits output
    in partition-major [128, tile, 90] f16 layout and the host un-permutes.

Sharding: pure data parallel, batch 2048 -> 8 cores x 256.
"""

import threading

import numpy as np
import ml_dtypes

import concourse.bass as bass
import concourse.mybir as mybir
import concourse.tile as tile
from concourse.vector_clock import ScopedClock
from concourse.bass_utils import run_bass_kernel_spmd

BF16 = ml_dtypes.bfloat16

# ---- problem constants (hardcoded per contract) ----
B, T, D_IN, H = 2048, 240, 180, 90
N_CORES = 8
BC = B // N_CORES          # 256 batches per core
NG = 270                   # live gate columns (i, g, o)

# ---- tiling config ----
NB = 16                    # batches per super-tile
ROWS = NB * T              # 3840 rows per super-tile
MT = ROWS // 128           # 30 m-tiles of 128 rows per super-tile
NSUP = BC // NB            # super-tiles per core
NMT = BC * T // 128        # 480 m-tiles per core

# ---- polynomial coefficients (minimax, see design notes) ----
# tanh(2y) ~= y*(P0 + P1*y^2) on |y|<=0.5, err < 4.6e-3 (3-op DVE variant)
_P3 = (1.95118309, -1.74855998)
# tanh(2y) ~= y*(P0 + P1*y^2 + P2*y^4) on |y|<=0.5, err < 3.9e-4
_P5 = (1.99432488, -2.46386409, 2.32965461)
# sigma(h)-0.5 ~= h*(Q0 + Q1*h^2) on |h|<=0.78, err < 3.5e-5
_Q = (0.24978061, -0.01935041)


# ---------------------------------------------------------------------------
# Walrus single-sync-wait workaround (installed walrus accepts at most ONE
# sync wait per ISA instruction; hoist overflow waits onto same-engine NOPs).
# ---------------------------------------------------------------------------
def _patched_lower_ordered_insts(self, ordered):
    for bb_name, insts in ordered.items():
        new = []
        for inst in insts:
            si = getattr(inst, "sync_info", None)
            if si is not None and len(si.on_wait) > 1:
                waits = list(si.on_wait)
                for w in waits[:-1]:
                    nop = mybir.InstNoOp(
                        name=self.nc.get_next_instruction_name(),
                        sync_info=mybir.SyncInfo(on_wait=[w], on_update=[]),
                        bass_nofuse=True,
                        engine=inst.engine,
                    )
                    new.append(nop)
                inst.sync_info = mybir.SyncInfo(
                    on_wait=[waits[-1]], on_update=list(si.on_update)
                )
            new.append(inst)
        insts[:] = new
    return _orig_lower_ordered_insts(self, ordered)


def _patched_drain_and_barrier(self, tick_clock, wait_clock):
    drain_inst = self.nc.sync.drain()
    wait_clock.add_sem_waits(
        drain_inst.ins, ScopedClock({None: tick_clock.global_clock})
    )
    si = drain_inst.ins.sync_info
    if si is not None and len(si.on_wait) > 1:
        waits = list(si.on_wait)
        drain_inst.ins.sync_info = mybir.SyncInfo(
            on_wait=waits[:1], on_update=list(si.on_update)
        )
        for i in range(1, len(waits)):
            nop = self.nc.sync.nop(nofuse=True)
            nop.ins.sync_info = mybir.SyncInfo(on_wait=[waits[i]], on_update=[])
    self.nc.all_engine_barrier()
    assert self.sems is not None
    popped = self.nc._tile_sem_poison_stack.pop()
    assert popped is self._sem_poison
    self.nc.clear_and_free_semaphores(list(self.sems.allocated().values()))
    self.nc.all_engine_barrier()


if not getattr(tile.TileContext, "_single_wait_patched", False):
    tile.TileContext._orig_lower_unpatched = tile.TileContext._lower_ordered_insts
    tile.TileContext._lower_ordered_insts = _patched_lower_ordered_insts
    tile.TileContext._drain_and_barrier = _patched_drain_and_barrier
    tile.TileContext._single_wait_patched = True
_orig_lower_ordered_insts = tile.TileContext._orig_lower_unpatched


def _groups_for(mt_j, gsz0):
    if gsz0 == 4:
        groups, left = [], mt_j
        while left >= 7 or left == 4:
            groups.append(4)
            left -= 4
        while left:
            groups.append(3)
            left -= 3
        return groups
    assert mt_j % gsz0 == 0
    return [gsz0] * (mt_j // gsz0)


def build_nc(
    out_dt="f16",
    bufs_x=3,
    bufs_t=3,
    bufs_ep=6,
    bufs_o=2,
    bufs_ps=2,
    ep_chunks=1,
    super_plan=None,
    gsz0=4,
    in_dma_chunks=2,
    tanh_dve_tiles=24,
    pool_c2=True,
    pool_out=True,
    stt_c2=True,
    tanh3=True,
    loop_repeat=None,
    w_dma_act=True,
    alloc_mode="stack",
    skip_ep=False,
    skip_act=False,
):
    f32 = mybir.dt.float32
    bf16 = mybir.dt.bfloat16
    fp8 = mybir.dt.float8e4
    odt = {"f32": f32, "bf16": bf16, "f16": mybir.dt.float16}[out_dt]
    FT = mybir.ActivationFunctionType

    nc = bass.Bass()
    # xq: x packed fp8 into two 91-row k-tiles (90 feature rows + a ones row
    # each); wq likewise holds W plus bias row fp8(b) in tile0 and the
    # residual b - fp8(b) in tile1, so the bias is exact to second order.
    xq = nc.declare_dram_parameter("xq", [91, 2, BC * T], fp8, isOutput=False)
    wq = nc.declare_dram_parameter("wq", [91, 2, NG], fp8, isOutput=False)
    # partition-major output: [partition, global m-tile, H]
    out = nc.declare_dram_parameter("out", [128, NMT, H], odt, isOutput=True)

    with tile.TileContext(nc, pool_alloc_mode=alloc_mode) as tc:
        with (
            tc.tile_pool(name="w", bufs=1) as wpool,
            tc.tile_pool(name="x", bufs=bufs_x) as xpool,
            tc.tile_pool(name="t", bufs=bufs_t) as tpool,
            tc.tile_pool(name="ep", bufs=bufs_ep) as eppool,
            tc.tile_pool(name="o", bufs=bufs_o) as opool,
            tc.tile_pool(name="ps", bufs=bufs_ps, space="PSUM") as pspool,
        ):
            wdma = nc.scalar if w_dma_act else nc.sync
            ws = wpool.tile([91, 2, NG], fp8)
            wdma.dma_start(ws[:], wq[:, :, :])

            if super_plan is None:
                plan = [(NB, ep_chunks)] * NSUP
            else:
                plan = super_plan
            assert sum(nb for nb, _ in plan) == BC
            max_rows = max(nb for nb, _ in plan) * T
            max_mt = max_rows // 128
            import contextlib

            loop_ctx = (
                tc.For_i(0, loop_repeat, 1)
                if loop_repeat is not None
                else contextlib.nullcontext()
            )
            def emit_chunks(Tt, mt_j, mt_base, epc_j):
                # staged S = sigma(0.5*G') per class: Si=sigma(i), Sg=sigma(2g),
                # So=sigma(o). Then c/2 = (Sg-0.5)*Si, U = tanh(c),
                # h = So*U, out = sigma(h).
                for h2 in range(epc_j if not skip_ep else 0):
                    mlo = mt_j * h2 // epc_j
                    mhi = mt_j * (h2 + 1) // epc_j
                    F = (mhi - mlo) * H
                    Ts = Tt[:, :, mlo:mhi, :]
                    Si = Ts[:, 0].rearrange("p t c -> p (t c)")
                    Sg = Ts[:, 1].rearrange("p t c -> p (t c)")
                    So = Ts[:, 2].rearrange("p t c -> p (t c)")

                    c2 = eppool.tile([128, F], bf16, tag="ep")
                    if pool_c2:
                        # Pool cannot run scalar_tensor_tensor (walrus
                        # rejects); split: Pool does the subtract, DVE the
                        # multiply.
                        tg = eppool.tile([128, F], bf16, tag="ep")
                        nc.gpsimd.tensor_scalar_add(tg[:], Sg, -0.5)
                        nc.vector.tensor_mul(c2[:], tg[:], Si)
                    elif stt_c2:
                        nc.vector.scalar_tensor_tensor(
                            c2[:], Sg, 0.5, Si,
                            mybir.AluOpType.subtract, mybir.AluOpType.mult,
                        )
                    else:
                        # ts@4x + tt@2x beats one stt@1x if perf modes engage
                        tg = eppool.tile([128, F], bf16, tag="ep")
                        nc.vector.tensor_scalar_add(tg[:], Sg, -0.5)
                        nc.vector.tensor_mul(c2[:], tg[:], Si)
                    U = eppool.tile([128, F], bf16, tag="ep")
                    # split U = tanh(2*c2): ACT (tanh table, scale=2) on the
                    # first tiles, DVE odd minimax poly on the rest,
                    # balancing the two engines
                    nmt_c = mhi - mlo
                    tdt = tanh_dve_tiles
                    kD = min((nmt_c * tdt) // MT, nmt_c) if tdt else 0
                    fA = (nmt_c - kD) * H
                    if fA > 0:
                        nc.scalar.activation(
                            U[:, 0:fA], c2[:, 0:fA], FT.Tanh, scale=2.0
                        )
                    if kD > 0:
                        Cs = c2[:, fA:F]
                        FD = F - fA
                        v = eppool.tile([128, FD], bf16, tag="pv")
                        nc.vector.tensor_mul(v[:], Cs, Cs)
                        if tanh3:
                            w = eppool.tile([128, FD], bf16, tag="pv")
                            nc.vector.tensor_scalar(
                                w[:], v[:], _P3[1], _P3[0],
                                mybir.AluOpType.mult, mybir.AluOpType.add,
                            )
                            nc.vector.tensor_mul(U[:, fA:F], w[:], Cs)
                        else:
                            w = eppool.tile([128, FD], bf16, tag="pv")
                            nc.vector.tensor_scalar(
                                w[:], v[:], _P5[2], _P5[1],
                                mybir.AluOpType.mult, mybir.AluOpType.add,
                            )
                            w2 = eppool.tile([128, FD], bf16, tag="pv")
                            nc.vector.tensor_mul(w2[:], w[:], v[:])
                            w3 = eppool.tile([128, FD], bf16, tag="pv")
                            nc.vector.tensor_scalar_add(w3[:], w2[:], _P5[0])
                            nc.vector.tensor_mul(U[:, fA:F], w3[:], Cs)
                    hT = eppool.tile([128, F], bf16, tag="ep")
                    nc.vector.tensor_mul(hT[:], So, U[:])

                    # sigma(h) = 0.5 + h*(Q0 + Q1*h^2), err < 3.5e-5
                    W2 = eppool.tile([128, F], bf16, tag="ep")
                    nc.vector.tensor_mul(W2[:], hT[:], hT[:])
                    V = eppool.tile([128, F], bf16, tag="ep")
                    nc.vector.tensor_scalar(
                        V[:], W2[:], _Q[1], _Q[0],
                        mybir.AluOpType.mult, mybir.AluOpType.add,
                    )
                    Z = eppool.tile([128, F], bf16, tag="ep")
                    nc.vector.tensor_mul(Z[:], V[:], hT[:])
                    OUT = opool.tile([128, F], odt)
                    (nc.gpsimd if pool_out else nc.vector).tensor_scalar_add(
                        OUT[:], Z[:], 0.5
                    )
                    nc.sync.dma_start(
                        out[:, mt_base + mlo : mt_base + mhi, :],
                        OUT[:].rearrange("p (t c) -> p t c", c=H),
                    )

            with loop_ctx:
                c0 = 0
                mt_base = 0
                # one-super software pipeline: emit super j's staging
                # (DMA + matmul + ACT T-pass), then super j-1's elementwise
                # chunks. In-order engines then never head-of-line block:
                # ACT streams [T(j), U(j-1)], DVE streams chunk ops whose
                # ACT inputs landed a super ago.
                staged_prev = None
                for j, (nb_j, epc_j) in enumerate(plan):
                    rows_j = nb_j * T
                    mt_j = rows_j // 128
                    xc = xpool.tile([91, 2, max_rows], fp8, tag="xc")
                    ndc = max(1, min(in_dma_chunks, mt_j))
                    for d in range(ndc):
                        rl = rows_j * d // ndc
                        rh = rows_j * (d + 1) // ndc
                        nc.sync.dma_start(
                            xc[:, :, rl:rh], xq[:, :, c0 + rl : c0 + rh]
                        )

                    # gate-class-major staging of tanh(0.5*G): [gate, mtile, col]
                    Tt = tpool.tile([128, 3, max_mt, H], bf16, tag="Tt")
                    groups = _groups_for(mt_j, gsz0)
                    m0 = 0
                    for gsz in groups:
                        ps = pspool.tile([128, gsz0, 512], f32, tag="ps")
                        for t in range(gsz):
                            m = m0 + t
                            nc.tensor.matmul(
                                ps[:, t, 0:NG],
                                xc[:, :, 128 * m : 128 * (m + 1)],
                                ws[:],
                                start=True,
                                stop=True,
                                perf_mode=mybir.MatmulPerfMode.DoubleRow,
                            )
                        if skip_act:
                            nc.vector.tensor_copy(
                                Tt[:, 0, m0 : m0 + gsz, :],
                                ps[:, 0:gsz, 0:H],
                            )
                        else:
                            nc.scalar.activation(
                                Tt[:, :, m0 : m0 + gsz, :].rearrange(
                                    "p g t c -> p t g c"
                                ),
                                ps[:, 0:gsz, 0:NG].rearrange(
                                    "p t (g c) -> p t g c", g=3
                                ),
                                FT.Sigmoid,
                                scale=0.5,
                            )
                        m0 += gsz

                    if staged_prev is not None:
                        emit_chunks(*staged_prev)
                    staged_prev = (Tt, mt_j, mt_base, epc_j)
                    c0 += rows_j
                    mt_base += mt_j
                if staged_prev is not None:
                    emit_chunks(*staged_prev)
    return nc


_cache = threading.local()


DEFAULT_PLAN = [(8, 1)] * 2 + [(16, 1)] * 14 + [(8, 1)] * 2
DEFAULT_CFG = dict(
    super_plan=DEFAULT_PLAN, tanh_dve_tiles=6, pool_c2=False, pool_out=False,
    stt_c2=False, bufs_t=4, bufs_ep=12, bufs_o=3, in_dma_chunks=1,
)


def _get_nc():
    nc = getattr(_cache, "nc", None)
    if nc is None:
        nc = build_nc(**DEFAULT_CFG)
        _cache.nc = nc
    return nc


FP8 = ml_dtypes.float8_e4m3


def _prep_inputs(x, W_ih, W_hh, b_ih, b_hh):
    x = np.asarray(x, dtype=np.float32)
    W = np.asarray(W_ih, dtype=np.float32)
    b = np.asarray(b_ih, dtype=np.float32) + np.asarray(b_hh, dtype=np.float32)
    # gate order [i, g, o]; prescale [2, 4, 2] so one sigma(0.5*G') ACT pass
    # yields [sigma(i), sigma(2g), sigma(o)] (tanh(g) = 2*sigma(2g) - 1)
    W_eff = np.concatenate(
        [2.0 * W[0:90], 4.0 * W[180:270], 2.0 * W[270:360]], axis=0
    )
    b_eff = np.concatenate(
        [2.0 * b[0:90], 4.0 * b[180:270], 2.0 * b[270:360]], axis=0
    )
    WT = W_eff.T                                  # (180, 270)
    wqv = np.zeros((91, 2, NG), dtype=FP8)
    wqv[0:90, 0] = WT[0:90].astype(FP8)
    wqv[0:90, 1] = WT[90:180].astype(FP8)
    b1 = b_eff.astype(FP8)
    wqv[90, 0] = b1
    wqv[90, 1] = (b_eff - b1.astype(np.float32)).astype(FP8)

    xr = x.reshape(B, D_IN, T).astype(FP8)        # (2048, 180, 240)
    # per-core pack: [k-row, tile, batch*col] planar, ones row at k=90
    xqs = []
    for c in range(N_CORES):
        xc = xr[c * BC : (c + 1) * BC]            # (256, 180, 240)
        xqv = np.empty((91, 2, BC * T), dtype=FP8)
        xqv[0:90, 0] = np.ascontiguousarray(
            xc[:, 0:90, :].transpose(1, 0, 2)
        ).reshape(90, BC * T)
        xqv[0:90, 1] = np.ascontiguousarray(
            xc[:, 90:180, :].transpose(1, 0, 2)
        ).reshape(90, BC * T)
        xqv[90, :] = 1.0
        xqs.append(xqv)
    return xqs, wqv


def kernel(x, W_ih, W_hh, b_ih, b_hh, _trace=False):
    xqs, wqv = _prep_inputs(x, W_ih, W_hh, b_ih, b_hh)
    nc = _get_nc()
    in_maps = [{"xq": xqs[c], "wq": wqv} for c in range(N_CORES)]
    res = run_bass_kernel_spmd(nc, in_maps, list(range(N_CORES)), trace=_trace)
    # device out: [128, NMT, H], row (128*g + p) -> un-permute on host
    outs = []
    for c in range(N_CORES):
        o = res.results[c]["out"].astype(np.float32)
        outs.append(o.transpose(1, 0, 2).reshape(BC, T, H))
    if _trace:
        kernel._last_results = res
    return np.concatenate(outs, axis=0)
